# revision 1
# baseline (speedup 1.0000x reference)
"""Frame-causal sparse attention block (LN + QKV + masked softmax attention
+ out-proj) on 8 TRN2 NeuronCores.

Sharding: core c handles batch b = c//2 and heads [4*(c%2), 4*(c%2)+4).
Per-core the kernel computes, fully on-chip:
  - LayerNorm statistics (mean/rstd) of its batch (LN affine folded into the
    QKV weights on host: W' = g (*) W plus rank-2 column-sum corrections)
  - Q^T, K^T in [channel, token] layout; V in [token, channel] layout with an
    appended ones-column so the A@V matmul also produces softmax denominators
  - block-sparse S^T = K @ Q^T at 128-token tile granularity (frame-causal
    mask: 64-token frames, queries attend to frames <= their own)
  - exp on ScalarE (scale 1/8 folded into the activation), A@V on PE,
    denominator division, out-projection (+ b_out/2 per core)
  - pairwise ReduceScatter (cores 2b, 2b+1) of the partial out-proj results
Host side only shards/transposes inputs and concatenates the output halves.

All matmuls run in float32r (full PE rate at moving dim >= 256). walrus
requires every f32r matmul operand's producer instruction to emit f32r:
DMA loads land in f32r tiles directly; on-chip producers (DVE copies/mults,
ACT exp) write f32r outputs. memset cannot produce f32r, reciprocal cannot
read PSUM, and no engine op may shift partition bases between operands.
"""

import sys

import numpy as np

sys.path.insert(0, "/opt/trn_rl_repo")

DIM = 512
HEADS = 8
DH = 64
INNER = 512
T = 2048
B = 4
EPS = 1e-5
NCORES = 8
HPC = 4  # heads per core
CQ = HPC * DH  # 256 channels per core for each of Q, K, V
NT = T // 128  # 16 token tiles
VSTRIDE = HPC * 128  # 512: per k-tile V block [h(64)|ones|zeros(63)] x 4, M=128 for FWL

_cache = {}


def _build_nc(stage=4, sub=9):
    from contextlib import ExitStack

    import concourse.bacc as bacc
    import concourse.bass as bass
    import concourse.tile as tile
    from concourse import mybir

    f32 = mybir.dt.float32
    f32r = mybir.dt.float32r
    bf16 = mybir.dt.bfloat16
    AF = mybir.ActivationFunctionType
    OP = mybir.AluOpType

    # Route every Exp/Ln activation to the one table set that contains both
    # (natural_log_exp_and_others): the default first-match pick splits them
    # across two sets and the per-division ln/exp chain then reloads ACT
    # tables ~24x (~31us). Stripping exp/ln from the other sets keeps list
    # indices (= act_func_set_id) intact.
    if not getattr(bacc, "_act_tables_patched", False):
        _orig_get_tables = bacc.get_activation_tables

        def _patched_get_tables(arch):
            tabs = _orig_get_tables(arch)
            both = [
                n
                for n, fns in tabs.items()
                if mybir.ActivationFunctionType.Exp in fns
                and mybir.ActivationFunctionType.Ln in fns
            ]
            if both:
                keep = both[0]
                tabs = {
                    n: (
                        fns
                        if n == keep
                        else fns
                        - {
                            mybir.ActivationFunctionType.Exp,
                            mybir.ActivationFunctionType.Ln,
                        }
                    )
                    for n, fns in tabs.items()
                }
            return tabs

        bacc.get_activation_tables = _patched_get_tables
        bacc._act_tables_patched = True

    nc = bacc.Bacc(
        "TRN2",
        target_bir_lowering=False,
        debug=False,
        num_devices=NCORES,
    )

    # ---- external I/O ----
    x_t = nc.dram_tensor("x_t", [DIM, T], bf16, kind="ExternalInput")
    # [512, 768] = [W'_q(256) | W'_k(256) | W'_v(256)] with LN-g folded in,
    # Q/K/V column blocks each ordered [h0|h1|h2|h3] x 64
    w_qkv_s = nc.dram_tensor("w_qkv_s", [DIM, 3 * CQ], bf16, kind="ExternalInput")
    # LN-fold corrections: csg[c] = sum_d g*W, csb[c] = sum_d b*W
    cs2 = nc.dram_tensor("cs2", [2, 3 * CQ], bf16, kind="ExternalInput")
    w_out_s = nc.dram_tensor("w_out_s", [2 * CQ, DIM], bf16, kind="ExternalInput")
    b_half = nc.dram_tensor("b_half", [1, DIM], bf16, kind="ExternalInput")
    out_ext = nc.dram_tensor("out", [T, DIM], bf16, kind="ExternalOutput")

    with tile.TileContext(nc) as tc:
        with ExitStack() as stack:
            constp = stack.enter_context(tc.tile_pool(name="const", bufs=1))
            work = stack.enter_context(tc.tile_pool(name="work", bufs=2))
            epool = stack.enter_context(tc.tile_pool(name="epool", bufs=3))
            small = stack.enter_context(tc.tile_pool(name="small", bufs=2))
            ps_misc = stack.enter_context(
                tc.tile_pool(name="ps_misc", bufs=2, space="PSUM")
            )
            ps_s = stack.enter_context(tc.tile_pool(name="ps_s", bufs=2, space="PSUM"))
            ps_o = stack.enter_context(tc.tile_pool(name="ps_o", bufs=1, space="PSUM"))
            dram = stack.enter_context(tc.tile_pool(name="dram", bufs=1, space="DRAM"))
            xspool = stack.enter_context(tc.tile_pool(name="xspool", bufs=1))
            xstack = stack.enter_context(ExitStack())
            xpool = xstack.enter_context(tc.tile_pool(name="xpool", bufs=1))

            # ---------- load inputs (f32r tiles straight from DMA) ----------
            xw = [
                xpool.tile([128, T], bf16, name=f"x{d}", tag=f"x{d}") for d in range(4)
            ]
            for d in range(4):
                for jx in range(4):
                    nc.sync.dma_start(
                        xw[d][:, jx * 512 : (jx + 1) * 512],
                        x_t[d * 128 : (d + 1) * 128, jx * 512 : (jx + 1) * 512],
                    )
            w_sb = [
                constp.tile([128, 3 * CQ], bf16, name=f"w{d}", tag=f"w{d}")
                for d in range(4)
            ]
            for d in range(4):
                nc.sync.dma_start(w_sb[d][:], w_qkv_s[d * 128 : (d + 1) * 128, :])
            cs_sb = constp.tile([2, 3 * CQ], bf16)
            nc.sync.dma_start(cs_sb[:], cs2[:])
            wout_sb = [
                constp.tile([128, DIM], bf16, name=f"wo{g}", tag=f"wo{g}")
                for g in range(4)
            ]
            for g in range(4):
                nc.sync.dma_start(wout_sb[g][:], w_out_s[g * 128 : (g + 1) * 128, :])
            brep = constp.tile([128, DIM], bf16)
            nc.sync.dma_start(brep[:], b_half[:].broadcast_to((128, DIM)))
            ones_bf = constp.tile([128, 128], bf16)
            nc.vector.memset(ones_bf[:], 1.0)

            zcols = constp.tile([128, 768], bf16)
            nc.vector.memset(zcols[:], 0.0)

            eps_col = constp.tile([128, 1], f32)
            nc.vector.memset(eps_col[:], EPS)

            # ---------- LN statistics ----------
            rstd_rep = xpool.tile([128, T], f32, name="rstd_rep", tag="rstd_rep")
            mu_rep = xpool.tile([128, T], f32, name="mu_rep", tag="mu_rep")
            corr2 = xspool.tile([2, T], bf16, name="corr2", tag="corr2")
            xs = [
                xspool.tile([128, T], bf16, name=f"xs{d}", tag=f"xs{d}")
                for d in range(4)
            ]
            # row1 must be all-ones; single-partition writes at base 1 are
            # illegal, so memset both rows and let the rmu writes below
            # overwrite row 0
            nc.vector.memset(corr2[:], 1.0)

            for j in range(4):
                cl = slice(j * 512, (j + 1) * 512)
                s1t = ps_misc.tile([128, 512], f32, name="s1t", tag="m")
                s2t = ps_misc.tile([128, 512], f32, name="s2t", tag="m")
                s1 = s1t[0:1, :]
                s2 = s2t[0:1, :]
                for d in range(4):
                    nc.tensor.matmul(
                        s1,
                        ones_bf[:, 0:1],
                        xw[d][:, cl],
                        start=(d == 0),
                        stop=(d == 3),
                    )
                for d in range(4):
                    xsq_c = work.tile([128, 512], bf16, name="xsq_c", tag="xsq_c")
                    nc.vector.tensor_tensor(
                        xsq_c[:], xw[d][:, cl], xw[d][:, cl], OP.mult
                    )
                    nc.tensor.matmul(
                        s2,
                        ones_bf[:, 0:1],
                        xsq_c[:],
                        start=(d == 0),
                        stop=(d == 3),
                    )
                sv1 = small.tile([1, 512], bf16, name="sv1", tag="sv1")
                sv2 = small.tile([1, 512], bf16, name="sv2", tag="sv2")
                nc.vector.tensor_copy(sv1[:], s1)
                nc.vector.tensor_copy(sv2[:], s2)
                rep1t = ps_s.tile([128, 1024], f32, name="rep1t", tag="s_ps")
                rep2t = ps_s.tile([128, 1024], f32, name="rep2t", tag="s_ps")
                rep1 = rep1t[:, 0:512]
                rep2 = rep2t[:, 0:512]
                nc.tensor.matmul(rep1, ones_bf[0:1, :], sv1[:])
                nc.tensor.matmul(rep2, ones_bf[0:1, :], sv2[:])
                # mu = s1/512 ; var = s2/512 - mu^2
                nc.vector.tensor_scalar(
                    mu_rep[:, cl], rep1, 1.0 / DIM, None, OP.mult
                )
                musq = work.tile([128, 512], f32, name="musq", tag="musq")
                nc.vector.tensor_tensor(
                    musq[:], mu_rep[:, cl], mu_rep[:, cl], OP.mult
                )
                var = work.tile([128, 512], f32, name="var", tag="var")
                nc.vector.scalar_tensor_tensor(
                    var[:], rep2, 1.0 / DIM, musq[:], OP.mult, OP.subtract
                )
                lnv = work.tile([128, 512], f32, name="lnv", tag="lnv")
                nc.scalar.activation(lnv[:], var[:], AF.Ln, bias=eps_col[:], scale=1.0)
                nc.scalar.activation(
                    rstd_rep[:, cl], lnv[:], AF.Exp, bias=0.0, scale=-0.5
                )
                nc.vector.scalar_tensor_tensor(
                    corr2[0:1, cl],
                    mu_rep[0:1, cl],
                    -1.0,
                    rstd_rep[0:1, cl],
                    OP.mult,
                    OP.mult,
                )
                for d in range(4):
                    nc.vector.tensor_tensor(
                        xs[d][:, cl], xw[d][:, cl], rstd_rep[:, cl], OP.mult
                    )

            # xs tiles are filled per-chunk inside the stats loop above
            if stage == 1:
                nc.sync.dma_start(out_ext[0:128, :].bitcast(f32r), xs[0][:, 0:512])
            xstack.close()  # x / mu / rstd space reused below
            qkvpool = stack.enter_context(tc.tile_pool(name="qkvpool", bufs=1))
            persist = stack.enter_context(tc.tile_pool(name="persist", bufs=1))

            if stage >= 2:
                # ---------- QKV projection ----------
                # Q^T, K^T: [128 (pair of heads), T]
                qT = [
                    qkvpool.tile([128, T], bf16, name=f"qT{p}", tag=f"qT{p}")
                    for p in range(2)
                ]
                kT = [
                    qkvpool.tile([128, T], bf16, name=f"kT{p}", tag=f"kT{p}")
                    for p in range(2)
                ]
                for ct in range(4):  # 0,1 -> Q pairs; 2,3 -> K pairs
                    dst = qT[ct] if ct < 2 else kT[ct - 2]
                    wcl = slice(ct * 128, (ct + 1) * 128)
                    for j in range(4):
                        cl = slice(j * 512, (j + 1) * 512)
                        acc = ps_misc.tile([128, 512], f32, name="qkv_ps", tag="m")
                        for d in range(4):
                            nc.tensor.matmul(
                                acc[:],
                                w_sb[d][:, wcl],
                                xs[d][:, cl],
                                start=(d == 0),
                                stop=False,
                            )
                        nc.tensor.matmul(
                            acc[:],
                            cs_sb[:, wcl],
                            corr2[:, cl],
                            start=False,
                            stop=True,
                        )
                        if (ct + j) % 2 == 0:
                            nc.vector.tensor_copy(dst[:, cl], acc[:])
                        else:
                            nc.scalar.activation(dst[:, cl], acc[:], AF.Copy)

                # V natural [token, channel] per-k-tile blocks with ones cols:
                # per tile t: cols [h*65 : h*65+64] = V_h, col h*65+64 = ones
                v_sb = qkvpool.tile([128, NT * VSTRIDE], bf16, name="v_sb", tag="v_sb")
                nc.vector.memset(v_sb[:], 0.0)
                ones_cols = v_sb[:].rearrange("p (t h c) -> p t h c", h=HPC, c=128)[
                    :, :, :, DH : DH + 1
                ]
                ones_src = ones_bf[:, 0:64].rearrange("p (t h c) -> p t h c", h=HPC, c=1)
                nc.vector.tensor_copy(ones_cols, ones_src)
                for tt in range(NT):
                    tl = slice(tt * 128, (tt + 1) * 128)
                    vact = ps_misc.tile([128, 512], f32, name="v_ps", tag="m")
                    vac = vact[:, 0:CQ]
                    for d in range(4):
                        nc.tensor.matmul(
                            vac,
                            xs[d][:, tl],
                            w_sb[d][:, 2 * CQ : 3 * CQ],
                            start=(d == 0),
                            stop=False,
                        )
                    nc.tensor.matmul(
                        vac,
                        corr2[:, tl],
                        cs_sb[:, 2 * CQ : 3 * CQ],
                        start=False,
                        stop=True,
                    )
                    dst = v_sb[:, tt * VSTRIDE : (tt + 1) * VSTRIDE].rearrange(
                        "p (h c) -> p h c", c=128
                    )[:, :, 0:DH]
                    nc.vector.tensor_copy(dst, vac.rearrange("p (h c) -> p h c", c=DH))

            if stage == 2:
                nc.sync.dma_start(out_ext[0:128, :].bitcast(f32r), qT[0][:, 0:512])
                nc.sync.dma_start(
                    out_ext[128:256, :].bitcast(f32r), v_sb[:, 0:512]
                )
            if stage >= 4:
                send = [
                    [
                        dram.tile(
                            [128, 512], bf16, name=f"send{p}_{j}", tag=f"send{p}_{j}"
                        )
                        for j in range(4)
                    ]
                    for p in range(2)
                ]
                recv = [
                    [
                        dram.tile(
                            [256, 512], bf16, name=f"recv{p}_{j}", tag=f"recv{p}_{j}"
                        )
                        for j in range(4)
                    ]
                    for p in range(2)
                ]
            if stage >= 3:
                # ---------- attention ----------
                # S^T = K @ Q^T per pair, 512-query chunks, k tiles of 128.
                onorm = [
                    [
                        persist.tile([64, T], bf16, name=f"on{p}{h}", tag=f"on{p}{h}")
                        for h in range(2)
                    ]
                    for p in range(2 if sub >= 3 else 0)
                ]
                onall = [
                    [
                        persist.tile(
                            [128, T], bf16, name=f"oa{p}{s}", tag=f"oa{p}{s}"
                        )
                        for s in range(2)
                    ]
                    for p in range(2 if stage >= 4 else 0)
                ]
                for p in range(2):
                    for j in range(4):  # query chunk [512j, 512j+512)
                        o_ps = [
                            ps_o.tile([128, 512], f32, name=f"o_ps{h}", tag=f"o_ps{h}")
                            for h in range(2)
                        ]
                        nkt = 4 * (j + 1)  # k tiles participating
                        for i in range(nkt):
                            q0 = max(512 * j, 128 * i)
                            n = 512 * (j + 1) - q0
                            off = q0 - 512 * j
                            diag = q0 == 128 * i
                            s_ps = ps_s.tile([128, 1024], f32, name="s_ps", tag="s_ps")
                            for h in range(2):
                                hr = slice(h * 64, (h + 1) * 64)
                                nc.tensor.matmul(
                                    s_ps[:, h * 512 + off : h * 512 + off + n],
                                    kT[p][hr, i * 128 : (i + 1) * 128],
                                    qT[p][hr, q0 : q0 + n],
                                )

                            e_sb = epool.tile([128, 1024], bf16, name="e_sb", tag="e_sb")
                            sr = s_ps[:].rearrange("p (x n) -> p x n", x=2)[
                                :, :, off : off + n
                            ]
                            er = e_sb[:].rearrange("p (x n) -> p x n", x=2)[
                                :, :, off : off + n
                            ]
                            nc.scalar.activation(er, sr, AF.Exp, bias=0.0, scale=0.125)
                            if off > 0:
                                # zero the never-computed query prefix so the
                                # A@V accumulation can run full-width
                                ez = e_sb[:].rearrange("p (x n) -> p x n", x=2)[
                                    :, :, 0:off
                                ]
                                zsrc = zcols[:, 0 : 2 * off].rearrange(
                                    "p (x n) -> p x n", x=2
                                )
                                nc.vector.tensor_copy(ez, zsrc)
                            if diag:
                                # frame-causal quadrant: rows 64:128 (frame
                                # 2i+1) must not see queries 128i..128i+64
                                eq = e_sb[64:128, :].rearrange(
                                    "p (x n) -> p x n", x=2
                                )[:, :, off : off + 64]
                                zq = zcols[64:128, 0:128].rearrange(
                                    "p (x n) -> p x n", x=2
                                )
                                nc.vector.tensor_copy(eq, zq)
                            first = i == 0
                            for h in range(2 if sub >= 2 else 0):
                                vblk = v_sb[
                                    :,
                                    i * VSTRIDE
                                    + (2 * p + h) * 128 : i * VSTRIDE
                                    + (2 * p + h + 1) * 128,
                                ]
                                nc.tensor.matmul(
                                    o_ps[h][:],
                                    vblk[:],
                                    e_sb[:, h * 512 : (h + 1) * 512],
                                    start=first,
                                    stop=(i == nkt - 1),
                                )
                        # normalize: rows 0:64 by reciprocal of row 64
                        # (denom). 1/x as exp(-ln(x)) on ScalarE (the DVE
                        # reciprocal ucode costs ~3.3us/row), batched over
                        # both heads; the partition replicate goes through a
                        # DRAM-bounce broadcast DMA so the PE never blocks
                        cl = slice(j * 512, (j + 1) * 512)
                        if sub >= 3:
                            oraw = small.tile([65, 1024], f32, name="oraw", tag="oraw")
                            for h in range(2):
                                nc.vector.tensor_copy(
                                    oraw[:, h * 512 : (h + 1) * 512],
                                    o_ps[h][0:65, :],
                                )
                            lnd = small.tile([65, 1024], f32, name="lnd", tag="lnd")
                            nc.scalar.activation(
                                lnd[64:65, :],
                                oraw[64:65, :],
                                AF.Ln,
                                bias=0.0,
                                scale=1.0,
                            )
                            rec = small.tile([65, 1024], f32, name="rec", tag="rec")
                            nc.scalar.activation(
                                rec[64:65, :],
                                lnd[64:65, :],
                                AF.Exp,
                                bias=0.0,
                                scale=-1.0,
                            )
                            rdb = dram.tile([1, 1024], f32, name="rdb", tag="rdb")
                            nc.sync.dma_start(rdb[:], rec[64:65, :])
                            rrep = small.tile([64, 1024], f32, name="rrep", tag="rrep")
                            nc.sync.dma_start(rrep[:], rdb[:].broadcast_to((64, 1024)))
                            for h in range(2):
                                nc.vector.tensor_tensor(
                                    onorm[p][h][:, cl],
                                    oraw[0:64, h * 512 : (h + 1) * 512],
                                    rrep[:, h * 512 : (h + 1) * 512],
                                    OP.mult,
                                )
                        if stage >= 4:
                            for h in range(2):
                                nc.sync.dma_start(
                                    send[p][j][h * 64 : (h + 1) * 64, :],
                                    onorm[p][h][:, cl],
                                )
                            nc.gpsimd.collective_compute(
                                "AllGather",
                                mybir.AluOpType.bypass,
                                replica_groups=[
                                    [2 * b, 2 * b + 1] for b in range(B)
                                ],
                                ins=[send[p][j][:].opt()],
                                outs=[recv[p][j][:].opt()],
                            )
                            for src_i in range(2):
                                nc.sync.dma_start(
                                    onall[p][src_i][:, cl],
                                    recv[p][j][src_i * 128 : (src_i + 1) * 128, :],
                                )

            if stage == 3 and sub >= 3:
                nc.sync.dma_start(
                    out_ext[0:64, :].bitcast(f32r), onorm[0][0][:, 0:512]
                )
            if stage >= 4:
                # out-projection: both cores compute the full T (pairwise
                # AllToAll is unsupported, so each pair AllGathers the
                # normalized heads in 512-token chunks as divisions finish,
                # and the host keeps the even core's output)
                for tt in range(NT):
                    tl = slice(tt * 128, (tt + 1) * 128)
                    jj = tt // 4
                    ops = ps_misc.tile([128, DIM], f32, name="out_ps", tag="m")
                    for p in range(2):
                        for src_i in range(2):
                            nc.tensor.matmul(
                                ops[:],
                                onall[p][src_i][:, tl],
                                wout_sb[2 * p + src_i][:],
                                start=(p == 0 and src_i == 0),
                                stop=(p == 1 and src_i == 1),
                            )
                    ostage = work.tile([128, DIM], bf16, name="ostage", tag="ostage")
                    nc.vector.tensor_tensor(ostage[:], ops[:], brep[:], OP.add)
                    nc.sync.dma_start(out_ext[tl, :], ostage[:])

    nc.compile()
    return nc


def _prep_in_maps(x, ln_g, ln_b, w_qkv, w_out, b_out):
    import ml_dtypes

    bf = ml_dtypes.bfloat16
    wp = ln_g[:, None] * w_qkv  # [512, 1536]
    csg = wp.sum(axis=0)  # [1536]
    csb = (ln_b[:, None] * w_qkv).sum(axis=0)
    in_maps = []
    for c in range(NCORES):
        b = c // 2
        heads = range(4 * (c % 2), 4 * (c % 2) + 4)
        qcols = np.concatenate([np.arange(h * DH, (h + 1) * DH) for h in heads])
        cols = np.concatenate([qcols, INNER + qcols, 2 * INNER + qcols])
        # w_out rows in recv order: [even pair0 (h0,h1), odd pair0 (h4,h5),
        # even pair1 (h2,h3), odd pair1 (h6,h7)]
        worder = [0, 1, 4, 5, 2, 3, 6, 7]
        wrows = np.concatenate([np.arange(h * DH, (h + 1) * DH) for h in worder])
        in_maps.append(
            {
                "x_t": np.ascontiguousarray(x[b].T).astype(bf),
                "w_qkv_s": np.ascontiguousarray(wp[:, cols]).astype(bf),
                "cs2": np.ascontiguousarray(np.stack([csg[cols], csb[cols]])).astype(
                    bf
                ),
                "w_out_s": np.ascontiguousarray(w_out[wrows, :]).astype(bf),
                "b_half": b_out.reshape(1, DIM).astype(bf),
            }
        )
    return in_maps


def _run(inputs, trace=False):
    from concourse.bass_utils import run_bass_kernel_spmd

    import os
    stage = int(os.environ.get("KSTAGE", "4"))
    sub = int(os.environ.get("KSUB", "9"))
    if ("nc", stage, sub) not in _cache:
        _cache[("nc", stage, sub)] = _build_nc(stage, sub)
    nc = _cache[("nc", stage, sub)]
    in_maps = _prep_in_maps(
        np.asarray(inputs["x"], dtype=np.float32),
        np.asarray(inputs["ln_g"], dtype=np.float32),
        np.asarray(inputs["ln_b"], dtype=np.float32),
        np.asarray(inputs["w_qkv"], dtype=np.float32),
        np.asarray(inputs["w_out"], dtype=np.float32),
        np.asarray(inputs["b_out"], dtype=np.float32),
    )
    res = run_bass_kernel_spmd(nc, in_maps, core_ids=list(range(NCORES)), trace=trace)
    out = np.empty((B, T, DIM), dtype=np.float32)
    for b in range(B):
        out[b] = res.results[2 * b]["out"].astype(np.float32)
    return out, res


def kernel(**inputs):
    return _run(inputs, trace=False)[0]


def kernel_traced(**inputs):
    out, res = _run(inputs, trace=True)
    return out, res

